# revision 44
# baseline (speedup 1.0000x reference)
"""Complex GRU cell on 8 Trainium2 NeuronCores (Bass/Tile).

Strategy
  - Data-parallel: batch 16384 -> 8 cores x 2048; 512x512 weights replicated.
  - Feature-major (transposed) layout on device: host pre-transposes x,h to
    [D, B_local] fp16 and pre-transposes the weights, so the kernel needs no
    on-device transposes and per-feature biases land on SBUF partitions.
  - Complex matmul uses the Gauss 3-multiplication trick: with host-prepared
    weight variants Wr, (Wi-Wr), -(Wr+Wi) and input sums (Xr+Xi),
      C = (Xr+Xi)@Wr, A = Xi@(-(Wr+Wi)), B = Xr@(Wi-Wr)
      re = C + A, im = C + B
  - Per wave the C group runs first and is copied to SBUF immediately (one
    ACT Copy), freeing its PSUM bank; the A/B groups then combine with bias
    via fused scalar_tensor_tensor on DVE: pre = (A + bias) + csb. PSUM tags
    are split R/Z vs C wave so R/Z buffers never wait on the C wave's
    (late) combine readers.
  - 1/|c| and |c| via Ln + two Exp LUT ops on m2=|c|^2+eps, batched per bc so
    the ACT table set switches twice per chunk; eps kills the |c|~0 NaN corner.
  - Software pipelining: the tanh+blend (beta) phase of chunk N-1 interleaves
    with the Z wave of chunk N; the final chunk balances its blend across
    DVE and GpSimd so the drain tail isn't Pool-bound.
  - DMAs are batched ([128,2048] tiles, partition-major APs on BOTH sides --
    a partition-middle SBUF AP here corrupts/races on real HW) and ordered so
    the first R-wave matmuls start ~3us in; weight tensors stream in behind.
  - Outputs packed re|im into one [128,1024] fp16 tile per (t4,bc) chunk.
  - rh products/sums run on the otherwise-idle GpSimd engine.
"""
import sys

for _p in ("/opt/trn_rl_repo",):
    if _p not in sys.path:
        sys.path.insert(0, _p)

import numpy as np
import concourse.bass as bass
import concourse.tile as tile
import concourse.mybir as mybir
from concourse.bass_utils import run_bass_kernel_spmd

F32, F16 = mybir.dt.float32, mybir.dt.float16
AF = mybir.ActivationFunctionType
ALU = mybir.AluOpType

RE, IM, IMN = 0, 1, 2  # weight variant slots: Wr, (Wi-Wr), -(Wr+Wi)
GZ, GR, GH = 0, 1, 2   # gates (z, r, candidate)

N_CORES = 8
B_FULL, D, H = 16384, 512, 512
B_LOCAL = B_FULL // N_CORES
BCHUNK = 512
NBC = B_LOCAL // BCHUNK

LAST_RUN_INFO = {}
_CACHE = {}


def _split_waits(nc, maxw=1):
    """walrus here allows 1 sync wait per instruction; hoist extras onto NoOps."""
    for fn in nc.m.functions:
        for bb in fn.blocks:
            out = []
            for inst in list(bb.instructions):
                si = inst.sync_info
                waits = list(si.on_wait) if si is not None else []
                if len(waits) > maxw:
                    extra, keep = waits[:-maxw], waits[-maxw:]
                    k = 0
                    while extra:
                        chunk, extra = extra[:maxw], extra[maxw:]
                        out.append(mybir.InstNoOp(
                            name=f"{inst.name}-wsplit{k}", engine=inst.engine,
                            ins=[], outs=[],
                            sync_info=mybir.SyncInfo(on_wait=chunk, on_update=[])))
                        k += 1
                    inst.sync_info = mybir.SyncInfo(on_wait=keep,
                                                    on_update=list(si.on_update))
                out.append(inst)
            bb.instructions[:] = out
    return nc


def _build(split_for_hw=True):
    nc = bass.Bass("TRN2", target_bir_lowering=False, debug=False)

    dram_acts = {}
    for nm in ("xs", "hs", "xi", "hi", "xr", "hr"):
        dram_acts[nm] = nc.dram_tensor(nm, [4, 128, B_LOCAL], F16,
                                       kind="ExternalInput")
    wx = nc.dram_tensor("wx", [3, 3, 4, 128, 512], F16, kind="ExternalInput")
    wh = nc.dram_tensor("wh", [3, 3, 4, 128, 512], F16, kind="ExternalInput")
    bias = nc.dram_tensor("bias", [128, 24], F32, kind="ExternalInput")
    # out[t4, p, bc, 0:512]=re, [..., 512:1024]=im
    out_d = nc.dram_tensor("out", [4, 128, 4, 1024], F16, kind="ExternalOutput")

    with tile.TileContext(nc) as tc:
        with (
            tc.tile_pool(name="wpool", bufs=1) as wpool,
            tc.tile_pool(name="apool", bufs=2) as apool,
            tc.tile_pool(name="rhpool", bufs=1) as rhpool,
            tc.tile_pool(name="zpool", bufs=2) as zpool,
            tc.tile_pool(name="spool", bufs=2) as spool,
            tc.tile_pool(name="opool", bufs=2) as opool,
            tc.tile_pool(name="pspool", bufs=1, space="PSUM") as pspool,
        ):
            W = {}      # (which, gate, variant) -> [128, 4dt*512] tile
            ACTT = {}   # (nm, bc) -> [128, 4dt*512] tile

            def dma_w(which, src, g, v):
                t = wpool.tile([128, 2048], F16, tag=f"w{which}{g}{v}")
                nc.sync.dma_start(
                    t[:].rearrange("p (d j) -> p d j", d=4),
                    src[g, v].rearrange("d p j -> p d j"))
                W[(which, g, v)] = t

            def dma_act(nm, bc):
                t = apool.tile([128, 2048], F16, tag=f"a{nm}")
                nc.sync.dma_start(
                    t[:].rearrange("p (d j) -> p d j", d=4),
                    dram_acts[nm][:, :, bc * BCHUNK:(bc + 1) * BCHUNK]
                    .rearrange("d p j -> p d j"))
                ACTT[(nm, bc)] = t

            def dma_act_half(nm, bc, half):
                t = ACTT[(nm, bc)]
                sl = slice(half * 2, half * 2 + 2)
                nc.sync.dma_start(
                    t[:, half * 1024:(half + 1) * 1024]
                    .rearrange("p (d j) -> p d j", d=2),
                    dram_acts[nm][sl, :, bc * BCHUNK:(bc + 1) * BCHUNK]
                    .rearrange("d p j -> p d j"))

            def dma_w_half(which, src, g, v, half):
                t = W[(which, g, v)]
                sl = slice(half * 2, half * 2 + 2)
                nc.sync.dma_start(
                    t[:, half * 1024:(half + 1) * 1024]
                    .rearrange("p (d j) -> p d j", d=2),
                    src[g, v, sl].rearrange("d p j -> p d j"))

            # DMA order: earliest-needed first, with the very first tensors
            # split in half so the first matmuls start ~3us in. R wave's C
            # group needs xs,hs + W(GR,RE); stream the rest behind it.
            ACTT[("xs", 0)] = apool.tile([128, 2048], F16, tag="axs",
                                         name="t_xs0")
            ACTT[("hs", 0)] = apool.tile([128, 2048], F16, tag="ahs",
                                         name="t_hs0")
            W[("x", GR, RE)] = wpool.tile([128, 2048], F16, tag="wxGRRE",
                                          name="t_wxgrre")
            W[("h", GR, RE)] = wpool.tile([128, 2048], F16, tag="whGRRE",
                                          name="t_whgrre")
            dma_act_half("xs", 0, 0); dma_w_half("x", wx, GR, RE, 0)
            dma_act_half("hs", 0, 0); dma_w_half("h", wh, GR, RE, 0)
            dma_act_half("xs", 0, 1); dma_w_half("x", wx, GR, RE, 1)
            dma_act_half("hs", 0, 1); dma_w_half("h", wh, GR, RE, 1)
            bt = wpool.tile([128, 24], F32, tag="bias")
            nc.sync.dma_start(bt[:], bias[:, :])
            dma_act("xi", 0); dma_act("hi", 0)
            dma_w("x", wx, GR, IMN); dma_w("h", wh, GR, IMN)
            dma_act("xr", 0); dma_act("hr", 0)
            dma_w("x", wx, GR, IM); dma_w("h", wh, GR, IM)
            for g in (GZ, GH):
                for v in (RE, IMN, IM):
                    dma_w("x", wx, g, v)
                    dma_w("h", wh, g, v)
            for nm in ("xs", "hs", "xi", "hi", "xr", "hr"):
                dma_act(nm, 1)

            def bslice(g, comp, t4):
                i = g * 8 + comp * 4 + t4
                return bt[:, i:i + 1]

            def mm_group(ps, g, v, srcs, t4, stop=True):
                """One Gauss product group accumulated into psum tile ps.
                srcs: list of ("x"|"h", tile_or_dict); dict is keyed by dt."""
                n = len(srcs) * 4
                i = 0
                for which, act in srcs:
                    for dt in range(4):
                        if isinstance(act, dict):
                            a = act[dt][:]
                        else:
                            a = act[:, dt * 512:(dt + 1) * 512]
                        nc.tensor.matmul(
                            ps[:],
                            W[(which, g, v)][:, dt * 512 + t4 * 128:
                                             dt * 512 + (t4 + 1) * 128],
                            a,
                            start=(i == 0),
                            stop=(stop and i == n - 1))
                        i += 1

            def wave(g, t4, xset, hset, tagp="pre", prebufs=2):
                """C/A/B groups for one (gate, t4) unit. Returns (pre_r, pre_i)
                fp16 tiles with bias folded in. The C wave rotates separate
                PSUM tags so R/Z-wave buffers never wait on the C wave's
                (late) combine/copy readers."""
                ctag = "psCb" if g == GH else "psCa"
                C = pspool.tile([128, BCHUNK], F32, tag=ctag,
                                bufs=1 if g == GH else 2)
                mm_group(C, g, RE, [("x", xset[0]), ("h", hset[0])], t4)
                csb = spool.tile([128, BCHUNK], F16, tag="csb")
                nc.scalar.activation(csb[:], C[:], AF.Copy)
                A = pspool.tile([128, BCHUNK], F32, tag="psAB", bufs=5)
                mm_group(A, g, IMN, [("x", xset[1]), ("h", hset[1])], t4)
                Bk = pspool.tile([128, BCHUNK], F32, tag="psAB", bufs=5)
                mm_group(Bk, g, IM, [("x", xset[2]), ("h", hset[2])], t4)
                pre_r = spool.tile([128, BCHUNK], F16, tag=f"{tagp}0",
                                   bufs=1 if tagp == "pre" else prebufs)
                pre_i = spool.tile([128, BCHUNK], F16, tag=f"{tagp}1",
                                   bufs=1 if tagp == "pre" else prebufs)
                nc.vector.scalar_tensor_tensor(
                    pre_r[:], A[:], bslice(g, 0, t4), csb[:], ALU.add, ALU.add)
                nc.vector.scalar_tensor_tensor(
                    pre_i[:], Bk[:], bslice(g, 1, t4), csb[:], ALU.add, ALU.add)
                return pre_r, pre_i

            # per-bc state carried into the next bc's R phase (software
            # pipeline: beta of bc-1 interleaves with R wave of bc)
            state = {}

            def beta_unit(bc, t4, drain=False):
                st = state[bc]
                cbr, cbi = st["cb"][t4]
                hr_t, hi_t = st["hr"], st["hi"]
                th = spool.tile([128, BCHUNK], F16, tag="th", bufs=1)
                nc.scalar.activation(th[:], st["mag"][t4][:], AF.Tanh)
                tf = spool.tile([128, BCHUNK], F16, tag="tf", bufs=4)
                nc.vector.tensor_tensor(tf[:], th[:], st["inv"][t4][:],
                                        ALU.mult)
                htr = spool.tile([128, BCHUNK], F16, tag="htr", bufs=1)
                hti = spool.tile([128, BCHUNK], F16, tag="hti", bufs=1)
                nc.vector.tensor_tensor(htr[:], tf[:], cbr[:], ALU.mult)
                nc.vector.tensor_tensor(hti[:], tf[:], cbi[:], ALU.mult)
                hrc = hr_t[:, t4 * 512:(t4 + 1) * 512]
                hic = hi_t[:, t4 * 512:(t4 + 1) * 512]
                dre = spool.tile([128, BCHUNK], F16, tag="dre", bufs=4)
                dim = spool.tile([128, BCHUNK], F16, tag="dim", bufs=4)
                nc.vector.tensor_tensor(dre[:], htr[:], hrc, ALU.subtract)
                nc.vector.tensor_tensor(dim[:], hti[:], hic, ALU.subtract)
                zr, zi = st["z"][(t4, 0)], st["z"][(t4, 1)]
                u1 = spool.tile([128, BCHUNK], F16, tag="u1", bufs=1)
                u2 = spool.tile([128, BCHUNK], F16, tag="u2", bufs=1)
                u3 = spool.tile([128, BCHUNK], F16, tag="u3", bufs=1)
                u4 = spool.tile([128, BCHUNK], F16, tag="u4", bufs=1)
                ere = spool.tile([128, BCHUNK], F16, tag="ere", bufs=1)
                eim = spool.tile([128, BCHUNK], F16, tag="eim", bufs=1)
                ot = opool.tile([128, 1024], F16, tag="out")
                # DVE handles the re lane; GpSimd the im lane (parallel)
                nc.vector.tensor_tensor(u1[:], zr[:], dre[:], ALU.mult)
                nc.vector.tensor_tensor(u2[:], zi[:], dim[:], ALU.mult)
                nc.vector.tensor_tensor(ere[:], u1[:], u2[:], ALU.subtract)
                nc.vector.tensor_tensor(ot[:, :512], hrc, ere[:], ALU.add)
                nc.gpsimd.tensor_tensor(u3[:], zr[:], dim[:], ALU.mult)
                nc.gpsimd.tensor_tensor(u4[:], zi[:], dre[:], ALU.mult)
                if drain:
                    # drain chunk: GpSimd ops are ~3.4x slower than DVE; keep
                    # its serial chain short so the tail isn't Pool-bound
                    nc.vector.tensor_tensor(eim[:], u3[:], u4[:], ALU.add)
                    nc.vector.tensor_tensor(ot[:, 512:], hic, eim[:], ALU.add)
                else:
                    nc.gpsimd.tensor_tensor(eim[:], u3[:], u4[:], ALU.add)
                    nc.gpsimd.tensor_tensor(ot[:, 512:], hic, eim[:], ALU.add)
                nc.sync.dma_start(out_d[t4, :, bc], ot[:])

            tf16 = {}

            def beta_pre(bc, t4):
                # z-independent part of the drain blend: tf = tanh(|c|)/|c|
                st = state[bc]
                th = spool.tile([128, BCHUNK], F16, tag="th", bufs=1)
                nc.scalar.activation(th[:], st["mag"][t4][:], AF.Tanh)
                tf = spool.tile([128, BCHUNK], F16, tag="tf", bufs=4)
                nc.vector.tensor_tensor(tf[:], th[:], st["inv"][t4][:],
                                        ALU.mult)
                cbr, cbi = st["cb"][t4]
                hr_t, hi_t = st["hr"], st["hi"]
                htr = spool.tile([128, BCHUNK], F16, tag="htr", bufs=1)
                hti = spool.tile([128, BCHUNK], F16, tag="hti", bufs=1)
                nc.vector.tensor_tensor(htr[:], tf[:], cbr[:], ALU.mult)
                nc.vector.tensor_tensor(hti[:], tf[:], cbi[:], ALU.mult)
                hrc = hr_t[:, t4 * 512:(t4 + 1) * 512]
                hic = hi_t[:, t4 * 512:(t4 + 1) * 512]
                dre = spool.tile([128, BCHUNK], F16, tag="dre", bufs=4)
                dim = spool.tile([128, BCHUNK], F16, tag="dim", bufs=4)
                nc.vector.tensor_tensor(dre[:], htr[:], hrc, ALU.subtract)
                nc.vector.tensor_tensor(dim[:], hti[:], hic, ALU.subtract)
                tf16[t4] = (dre, dim)

            def beta_post(bc, t4):
                st = state[bc]
                hr_t, hi_t = st["hr"], st["hi"]
                dre, dim = tf16[t4]
                hrc = hr_t[:, t4 * 512:(t4 + 1) * 512]
                hic = hi_t[:, t4 * 512:(t4 + 1) * 512]
                zr, zi = st["z"][(t4, 0)], st["z"][(t4, 1)]
                u1 = spool.tile([128, BCHUNK], F16, tag="u1", bufs=1)
                u2 = spool.tile([128, BCHUNK], F16, tag="u2", bufs=1)
                u3 = spool.tile([128, BCHUNK], F16, tag="u3", bufs=1)
                u4 = spool.tile([128, BCHUNK], F16, tag="u4", bufs=1)
                ere = spool.tile([128, BCHUNK], F16, tag="ere", bufs=1)
                eim = spool.tile([128, BCHUNK], F16, tag="eim", bufs=1)
                ot = opool.tile([128, 1024], F16, tag="out")
                nc.vector.tensor_tensor(u1[:], zr[:], dre[:], ALU.mult)
                nc.vector.tensor_tensor(u2[:], zi[:], dim[:], ALU.mult)
                nc.vector.tensor_tensor(ere[:], u1[:], u2[:], ALU.subtract)
                nc.vector.tensor_tensor(ot[:, :512], hrc, ere[:], ALU.add)
                nc.vector.tensor_tensor(u3[:], zr[:], dim[:], ALU.mult)
                nc.vector.tensor_tensor(u4[:], zi[:], dre[:], ALU.mult)
                nc.vector.tensor_tensor(eim[:], u3[:], u4[:], ALU.add)
                nc.vector.tensor_tensor(ot[:, 512:], hic, eim[:], ALU.add)
                nc.sync.dma_start(out_d[t4, :, bc], ot[:])

            for bc in range(NBC):
                xset = tuple(ACTT[(nm, bc)] for nm in ("xs", "xi", "xr"))
                hset = tuple(ACTT[(nm, bc)] for nm in ("hs", "hi", "hr"))
                hr_t = ACTT[("hr", bc)]
                hi_t = ACTT[("hi", bc)]

                # R wave: r = cv_sigmoid(...); rh = r*h; rhs = rhr+rhi.
                # beta units of the previous bc interleave between R units.
                rh_re, rh_im, rh_s = {}, {}, {}
                for t4 in range(4):
                    pre_r, pre_i = wave(GR, t4, xset, hset)
                    rr = spool.tile([128, BCHUNK], F16, tag="rr", bufs=1)
                    ri = spool.tile([128, BCHUNK], F16, tag="ri", bufs=1)
                    nc.scalar.activation(rr[:], pre_r[:], AF.Sigmoid)
                    nc.scalar.activation(ri[:], pre_i[:], AF.Sigmoid)
                    hrc = hr_t[:, t4 * 512:(t4 + 1) * 512]
                    hic = hi_t[:, t4 * 512:(t4 + 1) * 512]
                    t1 = spool.tile([128, BCHUNK], F16, tag="t1", bufs=1)
                    t2 = spool.tile([128, BCHUNK], F16, tag="t2", bufs=1)
                    t3 = spool.tile([128, BCHUNK], F16, tag="t3", bufs=1)
                    t4b = spool.tile([128, BCHUNK], F16, tag="t4b", bufs=1)
                    nc.vector.tensor_tensor(t1[:], rr[:], hrc, ALU.mult)
                    nc.vector.tensor_tensor(t2[:], ri[:], hic, ALU.mult)
                    nc.vector.tensor_tensor(t3[:], rr[:], hic, ALU.mult)
                    nc.vector.tensor_tensor(t4b[:], ri[:], hrc, ALU.mult)
                    rhr = rhpool.tile([128, BCHUNK], F16, tag=f"rhr{t4}")
                    rhi = rhpool.tile([128, BCHUNK], F16, tag=f"rhi{t4}")
                    rhs = rhpool.tile([128, BCHUNK], F16, tag=f"rhs{t4}")
                    rh_eng = nc.vector if bc == NBC - 1 else nc.gpsimd
                    rh_eng.tensor_tensor(rhr[:], t1[:], t2[:], ALU.subtract)
                    rh_eng.tensor_tensor(rhi[:], t3[:], t4b[:], ALU.add)
                    rh_eng.tensor_tensor(rhs[:], rhr[:], rhi[:], ALU.add)
                    rh_re[t4], rh_im[t4], rh_s[t4] = rhr, rhi, rhs

                # Z wave
                z16 = {}

                def z_unit(t4):
                    pre_r, pre_i = wave(GZ, t4, xset, hset)
                    for comp, pre in ((0, pre_r), (1, pre_i)):
                        zt = zpool.tile([128, BCHUNK], F16, tag=f"z{t4}{comp}")
                        nc.scalar.activation(zt[:], pre[:], AF.Sigmoid)
                        z16[(t4, comp)] = zt

                # C wave: c = px2 + (r*h)@Wh2^T (+bias)
                cb16, m16 = {}, {}
                inv16, mag16 = {}, {}
                state[bc] = {"cb": cb16, "z": z16, "inv": inv16, "mag": mag16,
                             "hr": hr_t, "hi": hi_t}

                def cmm_unit(t4):
                    # matmuls + csb + bias-combine only; sq/m2 issued later so
                    # the next csb isn't stuck behind them in the ACT queue
                    cb16[t4] = wave(GH, t4, (xset[0], xset[1], xset[2]),
                                    (rh_s, rh_im, rh_re), tagp="cb", prebufs=4)

                def sq_unit(t4):
                    cbr, cbi = cb16[t4]
                    sre = spool.tile([128, BCHUNK], F16, tag="sre", bufs=1)
                    sim_ = spool.tile([128, BCHUNK], F16, tag="sim", bufs=1)
                    nc.scalar.activation(sre[:], cbr[:], AF.Square)
                    nc.scalar.activation(sim_[:], cbi[:], AF.Square)
                    m2 = spool.tile([128, BCHUNK], F16, tag="m2", bufs=2)
                    nc.vector.scalar_tensor_tensor(
                        m2[:], sre[:], 1e-6, sim_[:], ALU.add, ALU.add)
                    m16[t4] = m2

                def alpha_unit(t4):
                    # mag = exp(0.5 ln m2), 1/mag = exp(-0.5 ln m2): all on
                    # ACT in the natural_log_exp table set (batched per bc to
                    # bound table switches); keeps th's deps on the ACT queue
                    lnm = spool.tile([128, BCHUNK], F16, tag="lnm", bufs=1)
                    nc.scalar.activation(lnm[:], m16[t4][:], AF.Ln)
                    mag = spool.tile([128, BCHUNK], F16, tag="mag", bufs=4)
                    nc.scalar.activation(mag[:], lnm[:], AF.Exp, scale=0.5)
                    inv = spool.tile([128, BCHUNK], F16, tag="inv", bufs=4)
                    nc.scalar.activation(inv[:], lnm[:], AF.Exp, scale=-0.5)
                    inv16[t4], mag16[t4] = inv, mag

                if bc < NBC - 1:
                    for t4 in range(4):
                        z_unit(t4)
                        if bc > 0:
                            beta_unit(bc - 1, t4)
                    # prefetch next-next batch chunk's activations
                    if bc + 2 < NBC:
                        for nm in ("xs", "hs", "xi", "hi", "xr", "hr"):
                            dma_act(nm, bc + 2)
                    for t4 in range(4):
                        cmm_unit(t4)
                        sq_unit(t4)
                    for t4 in range(4):
                        alpha_unit(t4)
                else:
                    # drain chunk: only Z unit 0 runs between the R and C
                    # waves; Z units 1-3 run AFTER the C wave so the
                    # alpha/tanh/blend chains overlap their matmuls and the
                    # post-matmul tail shrinks to one blend. The previous
                    # chunk's betas interleave with the C wave instead.
                    z_unit(0)
                    for t4 in range(4):
                        cmm_unit(t4)
                        sq_unit(t4)
                        beta_unit(bc - 1, t4)
                    for t4 in range(4):
                        alpha_unit(t4)
                    for t4 in range(4):
                        beta_pre(bc, t4)
                    beta_post(bc, 0)
                    for t4 in (1, 2, 3):
                        z_unit(t4)
                        beta_post(bc, t4)

    if split_for_hw:
        _split_waits(nc)
    return nc


def _prep(inputs):
    x_re, x_im = inputs["x_re"], inputs["x_im"]
    h_re, h_im = inputs["h_re"], inputs["h_im"]

    def actT(a, sl):
        return np.ascontiguousarray(
            a[sl].T.reshape(4, 128, B_LOCAL).astype(np.float16))

    def wvar(Wre, Wim):
        out = np.empty((3, 3, 4, 128, 512), np.float16)
        for g in range(3):
            WreT, WimT = Wre[g].T, Wim[g].T
            out[g, RE] = WreT.reshape(4, 128, 512)
            out[g, IM] = (WimT - WreT).reshape(4, 128, 512)
            out[g, IMN] = (-(WreT + WimT)).reshape(4, 128, 512)
        return out

    wxn = wvar(inputs["Wx_re"], inputs["Wx_im"])
    whn = wvar(inputs["Wh_re"], inputs["Wh_im"])
    # bias[p, g*8 + comp*4 + t4] = (bx+bh)[g, comp][t4*128+p]
    b_re = (inputs["bx_re"] + inputs["bh_re"])  # [3, 512]
    b_im = (inputs["bx_im"] + inputs["bh_im"])
    bias = np.empty((128, 24), np.float32)
    for g in range(3):
        for t4 in range(4):
            bias[:, g * 8 + 0 * 4 + t4] = b_re[g, t4 * 128:(t4 + 1) * 128]
            bias[:, g * 8 + 1 * 4 + t4] = b_im[g, t4 * 128:(t4 + 1) * 128]
    x_s = x_re + x_im
    h_s = h_re + h_im

    in_maps = []
    for c in range(N_CORES):
        sl = slice(c * B_LOCAL, (c + 1) * B_LOCAL)
        in_maps.append({
            "xr": actT(x_re, sl), "xi": actT(x_im, sl), "xs": actT(x_s, sl),
            "hr": actT(h_re, sl), "hi": actT(h_im, sl), "hs": actT(h_s, sl),
            "wx": wxn, "wh": whn, "bias": bias,
        })
    return in_maps


def kernel(**inputs):
    if "nc" not in _CACHE:
        nc = _build(split_for_hw=False)
        try:
            from concourse.timeline_sim import TimelineSim
            LAST_RUN_INFO["timeline_ns"] = int(TimelineSim(nc).simulate())
        except Exception:
            pass
        _CACHE["nc"] = _split_waits(nc)
    nc = _CACHE["nc"]

    in_maps = _prep(inputs)
    res = run_bass_kernel_spmd(nc, in_maps, list(range(N_CORES)))
    LAST_RUN_INFO["exec_time_ns"] = res.exec_time_ns

    out = np.empty((B_FULL, 512, 2), np.float32)
    for c, r in enumerate(res.results):
        o = r["out"].astype(np.float32)  # [4, 128, 4, 1024]
        sl = slice(c * B_LOCAL, (c + 1) * B_LOCAL)
        # o[t4, p, bc, 0:512] = re[t4*128+p, bc*512+j]
        re = o[:, :, :, :512].transpose(2, 3, 0, 1).reshape(B_LOCAL, 512)
        im = o[:, :, :, 512:].transpose(2, 3, 0, 1).reshape(B_LOCAL, 512)
        out[sl, :, 0] = re
        out[sl, :, 1] = im
    return out
